# revision 11
# baseline (speedup 1.0000x reference)
"""GrowingNeuralField message-passing kernel for 8 Trainium2 NeuronCores.

Sharding: each core owns 512 rows (i) of the 4096x4096 connection matrix,
stored in SBUF as 32 tiles of [128 j-partitions, 512 i-free] and never
materialized to DRAM.  conn[i,j] = exp(-dist/r_i) * (dist<maxr) *
(0.3 + 0.7*cos_sim) is built from two PE matmuls per tile:
  - dist^2 via a K=5 augmented matmul over centered positions
  - (0.3 + 0.7*cos_sim) via a K=33 augmented matmul over sqrt-scaled
    normalized features (bias term folded in as an extra K row)
Row sums come from ones-stationary matmuls; row-normalization is applied
to the aggregation *output* (per-i scale), which is mathematically
identical since aggregation is linear in conn.

Aggregation per iteration: out[b,i] = sum_j act[b,j]*conn[i,j] with
stationary act^T tiles [j,b] and moving conn tiles [j,i] (N=512 streams).
Activations are all-gathered (as [i,b] transposed slices) between
iterations; the final output contraction is computed per-core and the
[128,10] partials summed on host.
"""

import os
import sys

import numpy as np

for _p in ("/opt/trn_rl_repo",):
    if _p not in sys.path and os.path.isdir(_p):
        sys.path.insert(0, _p)

import concourse.bass as bass
import concourse.mybir as mybir
import concourse.tile as tile
from concourse import bacc
from concourse.bass_utils import run_bass_kernel_spmd
from concourse.masks import make_identity

N = 4096
IN_DIM = 784
IN_PAD = 896  # 7 * 128
OUT_DIM = 10
FEAT_DIM = 32
BATCH = 128
VOL = 100.0
CORES = 8
S = N // CORES  # 512 rows per core
JT = N // 128  # 32 j tiles
IC = S // 128  # 4 i chunks per core
KC = IN_PAD // 128  # 7 k chunks for the input matmul

F32 = mybir.dt.float32
# rows 0-32: sqrt-scaled features + bias row; rows 64-68: distance quintet
# (matmul operand base partition must be 0/32/64; a base-32 operand may only
# span 32 partitions, so the 33-row feature block sits at base 0)
AF_ROWS = 69
FB = 0   # feature block base row (33 rows)
DB = 64  # distance block base row (5 rows)


def _build(n_iter: int, maxr: float, neg_invr: float | None, trace: bool = False):
    """Trace the SPMD program. neg_invr is -1/(r+1e-6) when radii are
    uniform (folded into the Exp activation scale); None selects the
    general per-i path using a broadcast tile."""
    AluOp = mybir.AluOpType
    Act = mybir.ActivationFunctionType
    nc = bacc.Bacc(
        "TRN2", target_bir_lowering=False, debug=False, num_devices=CORES
    )

    xT3 = nc.dram_tensor("xT3", [128, KC, BATCH], F32, kind="ExternalInput")
    iwT3 = nc.dram_tensor("iwT3", [128, KC, S], F32, kind="ExternalInput")
    AFd = nc.dram_tensor("AF", [AF_ROWS, N], F32, kind="ExternalInput")
    AFod = nc.dram_tensor("AFo", [AF_ROWS, S], F32, kind="ExternalInput")
    wo3 = nc.dram_tensor("wo3", [128, IC, OUT_DIM], F32, kind="ExternalInput")
    thrd = nc.dram_tensor("thr1", [1, S], F32, kind="ExternalInput")
    invrd = None
    if neg_invr is None:
        invrd = nc.dram_tensor("invro", [1, S], F32, kind="ExternalInput")
    y_out = nc.dram_tensor("y_part", [BATCH, OUT_DIM], F32, kind="ExternalOutput")

    ag_in = [nc.dram_tensor(f"ag_in{k}", [S, BATCH], F32) for k in range(n_iter)]
    ag_out = [
        nc.dram_tensor(f"ag_out{k}", [N, BATCH], F32, addr_space="Shared")
        for k in range(n_iter)
    ]

    with tile.TileContext(nc) as tc:
        with (
            tc.tile_pool(name="consts", bufs=1) as consts,
            tc.tile_pool(name="conn", bufs=1) as connp,
            tc.tile_pool(name="acts", bufs=1) as acts,
            tc.tile_pool(name="work", bufs=3) as work,
            tc.tile_pool(name="psA", bufs=2, space="PSUM") as psA,
            tc.tile_pool(name="psB", bufs=2, space="PSUM") as psB,
            tc.tile_pool(name="ps1", bufs=1, space="PSUM") as ps1,
            tc.tile_pool(name="trbc", bufs=1, space="PSUM") as psM,
            tc.tile_pool(name="yps", bufs=1, space="PSUM") as psY,
        ):
            # ---- constant loads ----
            xT = consts.tile([128, KC, BATCH], F32, tag="xT")
            nc.sync.dma_start(out=xT[:], in_=xT3[:])
            iwT = consts.tile([128, KC, S], F32, tag="iwT")
            nc.sync.dma_start(out=iwT[:], in_=iwT3[:])
            AF = consts.tile([AF_ROWS, N], F32, tag="AF")
            nc.sync.dma_start(out=AF[:], in_=AFd[:])
            AFo = consts.tile([AF_ROWS, S], F32, tag="AFo")
            nc.sync.dma_start(out=AFo[:], in_=AFod[:])
            wo = consts.tile([128, IC, OUT_DIM], F32, tag="wo")
            nc.sync.dma_start(out=wo[:], in_=wo3[:])
            thr1 = consts.tile([1, S], F32, tag="thr1")
            nc.sync.dma_start(out=thr1[:], in_=thrd[:])

            ident = consts.tile([128, 128], F32, tag="ident")
            make_identity(nc, ident[:])
            ones_k = consts.tile([128, 1], F32, tag="ones_k")
            nc.gpsimd.memset(ones_k[:], 1.0)
            ones_m = consts.tile([1, 128], F32, tag="ones_m")
            nc.gpsimd.memset(ones_m[:], 1.0)

            # broadcast thresholds [1,S] -> [128,S]
            thr_b = consts.tile([128, S], F32, tag="thr_b")
            tb_ps = psM.tile([128, S], F32, tag="tr")
            nc.tensor.matmul(tb_ps[:], ones_m[:], thr1[:], start=True, stop=True)
            nc.scalar.copy(thr_b[:], tb_ps[:])

            invr_b = None
            if neg_invr is None:
                invro = consts.tile([1, S], F32, tag="invro")
                nc.sync.dma_start(out=invro[:], in_=invrd[:])
                invr_b = consts.tile([128, S], F32, tag="invr_b")
                iv_ps = psM.tile([128, S], F32, tag="tr")
                nc.tensor.matmul(iv_ps[:], ones_m[:], invro[:], start=True, stop=True)
                nc.scalar.copy(invr_b[:], iv_ps[:])

            # ---- act0 = (x @ iw'.T) for own i, [b, i] layout ----
            act_cur = acts.tile([BATCH, S], F32, tag="act_c")
            a0_ps = ps1.tile([BATCH, S], F32, tag="agg")
            for kc in range(KC):
                nc.tensor.matmul(
                    a0_ps[:],
                    xT[:, kc, :],
                    iwT[:, kc, :],
                    start=(kc == 0),
                    stop=(kc == KC - 1),
                )
            nc.vector.tensor_copy(act_cur[:], a0_ps[:])

            # ---- build conn tiles + accumulate row sums ----
            conn_t = []
            rs_ps = None
            if n_iter > 0:
                rs_ps = ps1.tile([1, S], F32, tag="rs")
                for jt in range(JT):
                    js = slice(jt * 128, (jt + 1) * 128)
                    d2 = psA.tile([128, S], F32, tag="d2")
                    nc.tensor.matmul(
                        d2[:], AF[DB : DB + 5, js], AFo[DB : DB + 5, :], start=True, stop=True
                    )
                    fps = psB.tile([128, S], F32, tag="f")
                    nc.tensor.matmul(
                        fps[:], AF[FB : FB + 33, js], AFo[FB : FB + 33, :],
                        start=True, stop=True,
                    )
                    d2c = work.tile([128, S], F32, tag="d2c")
                    nc.vector.tensor_scalar_max(d2c[:], d2[:], 1e-12)
                    dist = work.tile([128, S], F32, tag="dist")
                    nc.scalar.activation(dist[:], d2c[:], Act.Sqrt)
                    e = work.tile([128, S], F32, tag="e")
                    if neg_invr is not None:
                        nc.scalar.activation(e[:], dist[:], Act.Exp, scale=neg_invr)
                    else:
                        ds = work.tile([128, S], F32, tag="ds")
                        nc.vector.tensor_mul(ds[:], dist[:], invr_b[:])
                        nc.scalar.activation(e[:], ds[:], Act.Exp, scale=-1.0)
                    em = work.tile([128, S], F32, tag="em")
                    nc.vector.scalar_tensor_tensor(
                        em[:], dist[:], maxr, e[:], AluOp.is_lt, AluOp.mult
                    )
                    ct = connp.tile([128, S], F32, tag=f"conn{jt}")
                    nc.vector.tensor_mul(ct[:], em[:], fps[:])
                    conn_t.append(ct)
                    nc.tensor.matmul(
                        rs_ps[:], ones_k[:], ct[:],
                        start=(jt == 0), stop=(jt == JT - 1),
                    )

                # invsum broadcast tile [128, S]
                rs_sb = consts.tile([1, S], F32, tag="rs_sb")
                nc.vector.tensor_scalar_add(rs_sb[:], rs_ps[:], 1e-6)
                inv_sb = consts.tile([1, S], F32, tag="inv_sb")
                nc.vector.reciprocal(inv_sb[:], rs_sb[:])
                invs_b = consts.tile([128, S], F32, tag="invs_b")
                is_ps = psM.tile([128, S], F32, tag="tr")
                nc.tensor.matmul(is_ps[:], ones_m[:], inv_sb[:], start=True, stop=True)
                nc.scalar.copy(invs_b[:], is_ps[:])

            actT = acts.tile([128, JT, BATCH], F32, tag="actT")

            for k in range(n_iter):
                # transpose own slice [b, S] -> [S, b] and ship to the gather
                for ic in range(IC):
                    cs = slice(ic * 128, (ic + 1) * 128)
                    trp = psM.tile([128, 128], F32, tag="tr")
                    nc.tensor.transpose(trp[:], act_cur[:, cs], ident[:])
                    trs = work.tile([128, 128], F32, tag="trs")
                    nc.scalar.copy(trs[:], trp[:])
                    nc.sync.dma_start(out=ag_in[k][cs, :], in_=trs[:])
                nc.gpsimd.collective_compute(
                    "AllGather",
                    AluOp.bypass,
                    replica_groups=[list(range(CORES))],
                    ins=[ag_in[k].ap().opt()],
                    outs=[ag_out[k].ap().opt()],
                )
                # pull the gathered [N, b] back as 32 [128, b] tiles
                for g in range(4):
                    nc.sync.dma_start(
                        out=actT[:, g * 8 : (g + 1) * 8, :],
                        in_=ag_out[k][g * 1024 : (g + 1) * 1024, :].rearrange(
                            "(a p) b -> p a b", p=128
                        ),
                    )
                # aggregation: agg[b, i] = sum_j act[b, j] conn[i, j]
                agg = ps1.tile([BATCH, S], F32, tag="agg")
                for jt in range(JT):
                    nc.tensor.matmul(
                        agg[:], actT[:, jt, :], conn_t[jt][:],
                        start=(jt == 0), stop=(jt == JT - 1),
                    )
                # act = min(relu(act + agg/rowsum - thr), 100)
                u = work.tile([BATCH, S], F32, tag="u")
                nc.vector.scalar_tensor_tensor(
                    u[:], agg[:], 1.0, invs_b[:], AluOp.mult, AluOp.mult
                )
                v = work.tile([BATCH, S], F32, tag="v")
                nc.vector.scalar_tensor_tensor(
                    v[:], u[:], 1.0, act_cur[:], AluOp.mult, AluOp.add
                )
                w = work.tile([BATCH, S], F32, tag="w")
                nc.vector.tensor_sub(w[:], v[:], thr_b[:])
                act_nxt = acts.tile([BATCH, S], F32, tag=f"act{k + 1}")
                nc.vector.tensor_scalar(
                    act_nxt[:], w[:], 0.0, 100.0, AluOp.max, AluOp.min
                )
                act_cur = act_nxt

            # ---- output: y[b, o] = sum_own_i act[b, i] * wo'[i, o] ----
            y_ps = psY.tile([BATCH, OUT_DIM], F32, tag="y")
            for ic in range(IC):
                cs = slice(ic * 128, (ic + 1) * 128)
                trp = psM.tile([128, 128], F32, tag="tr")
                nc.tensor.transpose(trp[:], act_cur[:, cs], ident[:])
                a_sb = work.tile([128, 128], F32, tag="a_sb")
                nc.scalar.copy(a_sb[:], trp[:])
                nc.tensor.matmul(
                    y_ps[:], a_sb[:], wo[:, ic, :],
                    start=(ic == 0), stop=(ic == IC - 1),
                )
            y_sb = work.tile([BATCH, OUT_DIM], F32, tag="y_sb")
            nc.vector.tensor_copy(y_sb[:], y_ps[:])
            nc.sync.dma_start(out=y_out[:], in_=y_sb[:])

    nc.compile()
    return nc


_CACHE: dict = {}
LAST: dict = {}  # {"nc": ..., "in_maps": ...} from the most recent call


def kernel(**inputs) -> np.ndarray:
    x = np.ascontiguousarray(np.asarray(inputs["x"], dtype=np.float32))
    positions = np.asarray(inputs["positions"], dtype=np.float32)
    input_weights = np.asarray(inputs["input_weights"], dtype=np.float32)
    features = np.asarray(inputs["features"], dtype=np.float32)
    output_weights = np.asarray(inputs["output_weights"], dtype=np.float32)
    connection_radii = np.asarray(inputs["connection_radii"], dtype=np.float32)
    thresholds = np.asarray(inputs["thresholds"], dtype=np.float32)
    n_iter = int(np.asarray(inputs["n_iterations"]))

    # ---- host prep (all O(N * dim), float32 to match the fp32 reference) ----
    pos = np.clip(positions, np.float32(0.1), np.float32(VOL - 0.1))
    radii = np.clip(connection_radii, np.float32(1.0), np.float32(50.0))
    maxr = float(radii.max())
    uniform = bool(np.all(radii == radii[0]))
    neg_invr = -1.0 / (float(radii[0]) + 1e-6) if uniform else None
    invr_full = (
        None if uniform else (np.float32(1.0) / (radii + np.float32(1e-6)))
    )

    fn = features / np.maximum(
        np.linalg.norm(features, axis=1, keepdims=True), np.float32(1e-6)
    ).astype(np.float32)
    fn = fn.astype(np.float32)
    fa = np.concatenate(
        [
            np.sqrt(np.float32(0.7)) * fn.T,
            np.full((1, N), np.sqrt(np.float32(0.3)), dtype=np.float32),
        ],
        axis=0,
    )  # [33, N]

    pc = (pos - np.float32(VOL / 2)).astype(np.float32)  # centered
    nsq = (pc * pc).sum(axis=1, dtype=np.float32)
    A5 = np.stack(
        [pc[:, 0], pc[:, 1], pc[:, 2], nsq, np.ones(N, np.float32)], axis=0
    )  # stationary j rows
    B5 = np.stack(
        [
            np.float32(-2.0) * pc[:, 0],
            np.float32(-2.0) * pc[:, 1],
            np.float32(-2.0) * pc[:, 2],
            np.ones(N, np.float32),
            nsq,
        ],
        axis=0,
    )  # moving i rows
    AF = np.zeros((AF_ROWS, N), np.float32)
    AF[DB : DB + 5] = A5
    AF[FB : FB + 33] = fa
    BF = np.zeros((AF_ROWS, N), np.float32)
    BF[DB : DB + 5] = B5
    BF[FB : FB + 33] = fa

    xc = np.clip(pos[:, 0] / np.float32(VOL), np.float32(0.0), np.float32(1.0))
    inw = np.exp(np.float32(-3.0) * xc).astype(np.float32)
    inw = inw / (inw.sum(dtype=np.float32) + np.float32(1e-6))
    ow = np.exp(np.float32(3.0) * (xc - np.float32(1.0))).astype(np.float32)
    ow = ow / (ow.sum(dtype=np.float32) + np.float32(1e-6))

    iwp = (input_weights * inw[:, None]).astype(np.float32)  # [N, 784]
    iwpT = np.zeros((IN_PAD, N), np.float32)
    iwpT[:IN_DIM] = iwp.T
    xTp = np.zeros((IN_PAD, BATCH), np.float32)
    xTp[:IN_DIM] = x.T
    xT3 = np.ascontiguousarray(
        xTp.reshape(KC, 128, BATCH).transpose(1, 0, 2)
    )  # [128, KC, B]
    wop = (output_weights * ow[:, None]).astype(np.float32)  # [N, 10]

    key = (n_iter, maxr, neg_invr)
    if key not in _CACHE:
        _CACHE[key] = _build(n_iter, maxr, neg_invr)
    nc = _CACHE[key]

    in_maps = []
    for c in range(CORES):
        sl = slice(c * S, (c + 1) * S)
        m = {
            "xT3": xT3,
            "iwT3": np.ascontiguousarray(
                iwpT[:, sl].reshape(KC, 128, S).transpose(1, 0, 2)
            ),
            "AF": AF,
            "AFo": np.ascontiguousarray(BF[:, sl]),
            "wo3": np.ascontiguousarray(
                wop[sl].reshape(IC, 128, OUT_DIM).transpose(1, 0, 2)
            ),
            "thr1": np.ascontiguousarray(thresholds[sl].reshape(1, S)),
        }
        if not uniform:
            m["invro"] = np.ascontiguousarray(invr_full[sl].reshape(1, S))
        in_maps.append(m)

    LAST["nc"] = nc
    LAST["in_maps"] = in_maps
    res = run_bass_kernel_spmd(nc, in_maps, list(range(CORES)))
    y = np.zeros((BATCH, OUT_DIM), np.float32)
    for c in range(CORES):
        y += res.results[c]["y_part"]
    return y.astype(np.float32)


# revision 21
# speedup vs baseline: 1.5282x; 1.5282x over previous
"""GrowingNeuralField message-passing kernel for 8 Trainium2 NeuronCores.

Sharding: each core owns 512 rows (i) of the 4096x4096 connection matrix,
stored in SBUF as 32 tiles of [128 j-partitions, 512 i-free] and never
materialized to DRAM.  conn[i,j] = exp(-dist/r_i) * (dist<maxr) *
(0.3 + 0.7*cos_sim) is built from two PE matmuls per tile:
  - dist^2 via a K=5 augmented matmul over centered positions (fp32)
  - (0.3 + 0.7*cos_sim) via a K=33 augmented matmul over sqrt-scaled
    normalized features, bias folded in as an extra K row (fp32r)
Row sums come from ones-stationary fp32r matmuls; row-normalization is
applied to the aggregation *output* (per-i scale), which is identical
since aggregation is linear in conn.

Aggregation per iteration: agg[b,i] = sum_j act[b,j]*conn[i,j] with
stationary act^T tiles [j,b] and moving conn tiles [j,i] (N=512, fp32r).
Activations are all-gathered (as [i,b] transposed slices) between
iterations; act0 + the first gather are emitted ahead of the conn build
so the collective hides under it.  The final output contraction is
per-core, with [128,10] partials summed on host.

Engine split of the per-tile map build: PE d2/f/rowsum matmuls,
V max(d2,eps) + conn=em*f, S sqrt + exp (batched 16 tiles per
activation function to avoid ACT table reloads), GpSimd
em=(dist<maxr)*e.
"""

import os
import sys

import numpy as np

for _p in ("/opt/trn_rl_repo",):
    if _p not in sys.path and os.path.isdir(_p):
        sys.path.insert(0, _p)

import concourse.bass as bass
import concourse.mybir as mybir
import concourse.tile as tile
from concourse import bacc
from concourse.bass_utils import run_bass_kernel_spmd
from concourse.masks import make_identity

N = 4096
D2_SHIFT = 6e-3  # added to d^2 via the matmul so sqrt input is always > 0
IN_DIM = 784
IN_PAD = 896  # 7 * 128
OUT_DIM = 10
FEAT_DIM = 32
BATCH = 128
VOL = 100.0
CORES = 8
S = N // CORES  # 512 rows per core
JT = N // 128  # 32 j tiles
GRP = 16  # j tiles per activation-function batch
IC = S // 128  # 4 i chunks per core
KC = IN_PAD // 128  # 7 k chunks for the input matmul

F32 = mybir.dt.float32
F32R = mybir.dt.float32r
# feature operand: 33 rows (sqrt-scaled features + bias row), F32R
# distance operand: 5 rows (quintet), F32 — separate tensors so no
# partition-offset bitcasts are needed


def _r(ap):
    return ap.bitcast(F32R)


def _build(n_iter: int, mthr: float, neg_invr: float | None):
    """Trace the SPMD program. neg_invr is -1/(r+1e-6) when radii are
    uniform (folded into the Exp activation scale); None selects the
    general per-i path using a broadcast tile."""
    AluOp = mybir.AluOpType
    Act = mybir.ActivationFunctionType
    nc = bacc.Bacc(
        "TRN2", target_bir_lowering=False, debug=False, num_devices=CORES
    )

    xT3 = nc.dram_tensor("xT3", [128, KC, BATCH], F32, kind="ExternalInput")
    iwT3 = nc.dram_tensor("iwT3", [128, KC, S], F32, kind="ExternalInput")
    AFd = nc.dram_tensor("AF", [33, N], F32R, kind="ExternalInput")
    AFod = nc.dram_tensor("AFo", [33, S], F32R, kind="ExternalInput")
    D5d = nc.dram_tensor("D5", [5, N], F32, kind="ExternalInput")
    D5od = nc.dram_tensor("D5o", [5, S], F32, kind="ExternalInput")
    wo3 = nc.dram_tensor("wo3", [128, IC, OUT_DIM], F32, kind="ExternalInput")
    thrd = nc.dram_tensor("thr1", [1, S], F32, kind="ExternalInput")
    invrd = None
    if neg_invr is None:
        invrd = nc.dram_tensor("invro", [1, S], F32, kind="ExternalInput")
    y_out = nc.dram_tensor("y_part", [BATCH, OUT_DIM], F32, kind="ExternalOutput")

    ag_in = [nc.dram_tensor(f"ag_in{k}", [S, BATCH], F32R) for k in range(n_iter)]
    ag_out = [
        nc.dram_tensor(f"ag_out{k}", [N, BATCH], F32R, addr_space="Shared")
        for k in range(n_iter)
    ]

    with tile.TileContext(nc) as tc:
        with (
            tc.tile_pool(name="consts", bufs=1) as consts,
            tc.tile_pool(name="conn", bufs=1) as connp,
            tc.tile_pool(name="acts", bufs=1) as acts,
            tc.tile_pool(name="dists", bufs=1) as distp,
            tc.tile_pool(name="work", bufs=3) as work,
            tc.tile_pool(name="small", bufs=1) as small,
            tc.tile_pool(name="psA", bufs=2, space="PSUM") as psA,
            tc.tile_pool(name="psB", bufs=2, space="PSUM") as psB,
            tc.tile_pool(name="ps1", bufs=1, space="PSUM") as ps1,
            tc.tile_pool(name="ptr", bufs=1, space="PSUM") as ptr,
            tc.tile_pool(name="bcy", bufs=1, space="PSUM") as bcy,
        ):
            # ---- constant loads ----
            xT = consts.tile([128, KC, BATCH], F32, tag="xT")
            nc.sync.dma_start(out=xT[:], in_=xT3[:])
            iwT = consts.tile([128, KC, S], F32, tag="iwT")
            nc.sync.dma_start(out=iwT[:], in_=iwT3[:])
            AF = consts.tile([33, N], F32R, tag="AF")
            nc.sync.dma_start(out=AF[:], in_=AFd[:])
            AFo = consts.tile([33, S], F32R, tag="AFo")
            nc.sync.dma_start(out=AFo[:], in_=AFod[:])
            D5 = consts.tile([5, N], F32, tag="D5")
            nc.sync.dma_start(out=D5[:], in_=D5d[:])
            D5o = consts.tile([5, S], F32, tag="D5o")
            nc.sync.dma_start(out=D5o[:], in_=D5od[:])
            wo = consts.tile([128, IC, OUT_DIM], F32, tag="wo")
            nc.sync.dma_start(out=wo[:], in_=wo3[:])
            thr1 = consts.tile([1, S], F32, tag="thr1")
            nc.sync.dma_start(out=thr1[:], in_=thrd[:])

            ident = consts.tile([128, 128], F32, tag="ident")
            make_identity(nc, ident[:])
            ones_kf = consts.tile([128, 1], F32, tag="ones_kf")
            nc.gpsimd.memset(ones_kf[:], 1.0)
            ones_k = consts.tile([128, 1], F32R, tag="ones_k")
            nc.vector.tensor_copy(ones_k[:], ones_kf[:])
            ones_m = consts.tile([1, 128], F32, tag="ones_m")
            nc.gpsimd.memset(ones_m[:], 1.0)

            # broadcast thresholds [1,S] -> [128,S]
            thr_b = consts.tile([128, S], F32, tag="thr_b")
            tb_ps = bcy.tile([128, S], F32, tag="bc")
            nc.tensor.matmul(tb_ps[:], ones_m[:], thr1[:], start=True, stop=True)
            nc.scalar.copy(thr_b[:], tb_ps[:])

            invr_b = None
            if neg_invr is None:
                invro = consts.tile([1, S], F32, tag="invro")
                nc.sync.dma_start(out=invro[:], in_=invrd[:])
                invr_b = consts.tile([128, S], F32, tag="invr_b")
                iv_ps = bcy.tile([128, S], F32, tag="bc")
                nc.tensor.matmul(iv_ps[:], ones_m[:], invro[:], start=True, stop=True)
                nc.scalar.copy(invr_b[:], iv_ps[:])

            # ---- act0 = (x @ iw'.T) for own i, [b, i] layout ----
            act_cur = acts.tile([BATCH, S], F32, tag="act_c")
            a0_ps = ps1.tile([BATCH, S], F32, tag="agg")
            for kc in range(KC):
                nc.tensor.matmul(
                    a0_ps[:],
                    xT[:, kc, :],
                    iwT[:, kc, :],
                    start=(kc == 0),
                    stop=(kc == KC - 1),
                )
            nc.vector.tensor_copy(act_cur[:], a0_ps[:])

            actT = acts.tile([128, JT, BATCH], F32R, tag="actT")

            def gather(k, act_sb):
                """Transpose own [b,S] slice to [S,b], AllGather to full
                [N,b], land as 32 [128,b] stationary tiles."""
                for ic in range(IC):
                    cs = slice(ic * 128, (ic + 1) * 128)
                    trp = ptr.tile([128, 128], F32, tag="tr")
                    nc.tensor.transpose(trp[:], act_sb[:, cs], ident[:])
                    trs = work.tile([128, 128], F32R, tag="trs")
                    nc.vector.tensor_copy(trs[:], trp[:])
                    nc.sync.dma_start(out=ag_in[k][cs, :], in_=trs[:])
                nc.gpsimd.collective_compute(
                    "AllGather",
                    AluOp.bypass,
                    replica_groups=[list(range(CORES))],
                    ins=[ag_in[k].ap().opt()],
                    outs=[ag_out[k].ap().opt()],
                )
                for g in range(4):
                    nc.sync.dma_start(
                        out=actT[:, g * 8 : (g + 1) * 8, :],
                        in_=ag_out[k][g * 1024 : (g + 1) * 1024, :].rearrange(
                            "(a p) b -> p a b", p=128
                        ),
                    )

            if n_iter > 0:
                gather(0, act_cur)

            # ---- build conn tiles + accumulate row sums ----
            conn_t = {}
            dist_t = {}
            rs_ps = None
            if n_iter > 0:
                rs_ps = ps1.tile([1, S], F32, tag="rs")
                for g0 in range(0, JT, GRP):
                    # pass A: dist = sqrt(max(d2, 1e-12))  (one Sqrt batch)
                    for jt in range(g0, g0 + GRP):
                        js = slice(jt * 128, (jt + 1) * 128)
                        d2 = psA.tile([128, S], F32, tag="d2")
                        nc.tensor.matmul(
                            d2[:], D5[:, js], D5o[:, :],
                            start=True, stop=True,
                        )
                        dt_ = distp.tile([128, S], F32, tag=f"dist{jt % GRP}")
                        nc.scalar.activation(dt_[:], d2[:], Act.Sqrt)
                        dist_t[jt] = dt_
                    # pass B: e = exp(-dist/r); conn = (dist<maxr)*e*f
                    for jt in range(g0, g0 + GRP):
                        js = slice(jt * 128, (jt + 1) * 128)
                        dt_ = dist_t[jt]
                        fps = psB.tile([128, S], F32, tag="f")
                        nc.tensor.matmul(
                            fps[:], AF[:, js], AFo[:, :],
                            start=True, stop=True,
                        )
                        e = work.tile([128, S], F32, tag="e")
                        if neg_invr is not None:
                            nc.scalar.activation(e[:], dt_[:], Act.Exp, scale=neg_invr)
                        else:
                            nc.vector.tensor_mul(e[:], dt_[:], invr_b[:])
                            nc.scalar.activation(e[:], e[:], Act.Exp, scale=-1.0)
                        em = work.tile([128, S], F32, tag="em")
                        nc.vector.scalar_tensor_tensor(
                            em[:], dt_[:], mthr, e[:], AluOp.is_lt, AluOp.mult
                        )
                        ct = connp.tile([128, S], F32R, tag=f"conn{jt}")
                        nc.vector.tensor_mul(ct[:], em[:], fps[:])
                        conn_t[jt] = ct
                        nc.tensor.matmul(
                            rs_ps[:], ones_k[:], ct[:],
                            start=(jt == 0), stop=(jt == JT - 1),
                        )

                # invsum broadcast tile [128, S]
                rs_sb = consts.tile([1, S], F32, tag="rs_sb")
                nc.vector.tensor_scalar_add(rs_sb[:], rs_ps[:], 1e-6)
                inv_sb = consts.tile([1, S], F32, tag="inv_sb")
                nc.vector.reciprocal(inv_sb[:], rs_sb[:])
                invs_b = consts.tile([128, S], F32, tag="invs_b")
                is_ps = bcy.tile([128, S], F32, tag="bc")
                nc.tensor.matmul(is_ps[:], ones_m[:], inv_sb[:], start=True, stop=True)
                nc.scalar.copy(invs_b[:], is_ps[:])

            for k in range(n_iter):
                # aggregation: agg[b, i] = sum_j act[b, j] conn[i, j]
                agg = ps1.tile([BATCH, S], F32, tag="agg")
                for jt in range(JT):
                    nc.tensor.matmul(
                        agg[:], actT[:, jt, :], conn_t[jt][:],
                        start=(jt == 0), stop=(jt == JT - 1),
                    )
                # act = min(relu(act + agg/rowsum - thr), 100)
                u = small.tile([BATCH, S], F32, tag="u")
                nc.vector.scalar_tensor_tensor(
                    u[:], agg[:], 1.0, invs_b[:], AluOp.mult, AluOp.mult
                )
                v = small.tile([BATCH, S], F32, tag="v")
                nc.vector.scalar_tensor_tensor(
                    v[:], u[:], 1.0, act_cur[:], AluOp.mult, AluOp.add
                )
                w = small.tile([BATCH, S], F32, tag="w")
                nc.vector.tensor_sub(w[:], v[:], thr_b[:])
                act_nxt = acts.tile([BATCH, S], F32, tag=f"act{k + 1}")
                nc.vector.tensor_scalar(
                    act_nxt[:], w[:], 0.0, 100.0, AluOp.max, AluOp.min
                )
                act_cur = act_nxt
                if k + 1 < n_iter:
                    gather(k + 1, act_cur)

            # ---- output: y[b, o] = sum_own_i act[b, i] * wo'[i, o] ----
            y_ps = bcy.tile([BATCH, OUT_DIM], F32, tag="bc")
            for ic in range(IC):
                cs = slice(ic * 128, (ic + 1) * 128)
                trp = ptr.tile([128, 128], F32, tag="tr")
                nc.tensor.transpose(trp[:], act_cur[:, cs], ident[:])
                a_sb = small.tile([128, 128], F32, tag="a_sb")
                nc.scalar.copy(a_sb[:], trp[:])
                nc.tensor.matmul(
                    y_ps[:], a_sb[:], wo[:, ic, :],
                    start=(ic == 0), stop=(ic == IC - 1),
                )
            y_sb = small.tile([BATCH, OUT_DIM], F32, tag="y_sb")
            nc.vector.tensor_copy(y_sb[:], y_ps[:])
            nc.sync.dma_start(out=y_out[:], in_=y_sb[:])

    nc.compile()
    return nc


_CACHE: dict = {}
LAST: dict = {}  # {"nc": ..., "in_maps": ...} from the most recent call


def kernel(**inputs) -> np.ndarray:
    x = np.ascontiguousarray(np.asarray(inputs["x"], dtype=np.float32))
    positions = np.asarray(inputs["positions"], dtype=np.float32)
    input_weights = np.asarray(inputs["input_weights"], dtype=np.float32)
    features = np.asarray(inputs["features"], dtype=np.float32)
    output_weights = np.asarray(inputs["output_weights"], dtype=np.float32)
    connection_radii = np.asarray(inputs["connection_radii"], dtype=np.float32)
    thresholds = np.asarray(inputs["thresholds"], dtype=np.float32)
    n_iter = int(np.asarray(inputs["n_iterations"]))

    # ---- host prep (all O(N * dim), float32 to match the fp32 reference) ----
    pos = np.clip(positions, np.float32(0.1), np.float32(VOL - 0.1))
    radii = np.clip(connection_radii, np.float32(1.0), np.float32(50.0))
    maxr = float(radii.max())
    uniform = bool(np.all(radii == radii[0]))
    neg_invr = -1.0 / (float(radii[0]) + 1e-6) if uniform else None
    invr_full = (
        None if uniform else (np.float32(1.0) / (radii + np.float32(1e-6)))
    )

    fn = features / np.maximum(
        np.linalg.norm(features, axis=1, keepdims=True), np.float32(1e-6)
    ).astype(np.float32)
    fn = fn.astype(np.float32)
    fa = np.concatenate(
        [
            np.sqrt(np.float32(0.7)) * fn.T,
            np.full((1, N), np.sqrt(np.float32(0.3)), dtype=np.float32),
        ],
        axis=0,
    )  # [33, N]

    pc = (pos - np.float32(VOL / 2)).astype(np.float32)  # centered
    nsq = (pc * pc).sum(axis=1, dtype=np.float32)
    A5 = np.stack(
        [pc[:, 0], pc[:, 1], pc[:, 2], nsq, np.ones(N, np.float32)], axis=0
    )  # stationary j rows
    A5 = np.ascontiguousarray(A5)
    B5 = np.stack(
        [
            np.float32(-2.0) * pc[:, 0],
            np.float32(-2.0) * pc[:, 1],
            np.float32(-2.0) * pc[:, 2],
            np.ones(N, np.float32),
            nsq + np.float32(D2_SHIFT),
        ],
        axis=0,
    )  # moving i rows
    B5 = np.ascontiguousarray(B5)
    AF = np.ascontiguousarray(fa)

    xc = np.clip(pos[:, 0] / np.float32(VOL), np.float32(0.0), np.float32(1.0))
    inw = np.exp(np.float32(-3.0) * xc).astype(np.float32)
    inw = inw / (inw.sum(dtype=np.float32) + np.float32(1e-6))
    ow = np.exp(np.float32(3.0) * (xc - np.float32(1.0))).astype(np.float32)
    ow = ow / (ow.sum(dtype=np.float32) + np.float32(1e-6))

    iwp = (input_weights * inw[:, None]).astype(np.float32)  # [N, 784]
    iwpT = np.zeros((IN_PAD, N), np.float32)
    iwpT[:IN_DIM] = iwp.T
    xTp = np.zeros((IN_PAD, BATCH), np.float32)
    xTp[:IN_DIM] = x.T
    xT3 = np.ascontiguousarray(
        xTp.reshape(KC, 128, BATCH).transpose(1, 0, 2)
    )  # [128, KC, B]
    wop = (output_weights * ow[:, None]).astype(np.float32)  # [N, 10]

    import math
    mthr = float(np.float32(math.sqrt(maxr * maxr + D2_SHIFT)))
    key = (n_iter, mthr, neg_invr)
    if key not in _CACHE:
        _CACHE[key] = _build(n_iter, mthr, neg_invr)
    nc = _CACHE[key]

    in_maps = []
    for c in range(CORES):
        sl = slice(c * S, (c + 1) * S)
        m = {
            "xT3": xT3,
            "iwT3": np.ascontiguousarray(
                iwpT[:, sl].reshape(KC, 128, S).transpose(1, 0, 2)
            ),
            "AF": AF,
            "AFo": np.ascontiguousarray(fa[:, sl]),
            "D5": A5,
            "D5o": np.ascontiguousarray(B5[:, sl]),
            "wo3": np.ascontiguousarray(
                wop[sl].reshape(IC, 128, OUT_DIM).transpose(1, 0, 2)
            ),
            "thr1": np.ascontiguousarray(thresholds[sl].reshape(1, S)),
        }
        if not uniform:
            m["invro"] = np.ascontiguousarray(invr_full[sl].reshape(1, S))
        in_maps.append(m)

    LAST["nc"] = nc
    LAST["in_maps"] = in_maps
    res = run_bass_kernel_spmd(nc, in_maps, list(range(CORES)))
    y = np.zeros((BATCH, OUT_DIM), np.float32)
    for c in range(CORES):
        y += res.results[c]["y_part"]
    return y.astype(np.float32)
